# revision 1
# baseline (speedup 1.0000x reference)
"""nn_MultiHeadAttention kernel for 8 Trainium2 NeuronCores.

Sharding: 8 cores = 4 batches (data parallel) x 2 head-groups of 8 heads
(tensor parallel). Each core computes its batch's QKV projection for its
head group (column-parallel), RoPE, causal attention, and a partial
out-projection (row-parallel). Host sums the two partials per batch and
adds the output bias.

Per-core dataflow (all matmuls float32r, N>=256 chunks):
  Phase 1: xT resident in SBUF. q,k produced transposed [D,T] (bias fused
    into the PSUM eviction on DVE), v produced untransposed [T,D] (bias via
    a K=1 matmul of ones x bv). All spilled to DRAM scratch.
  Phase 2 (per head): RoPE on q,k (4 DVE ops via the swap-halves trick),
    scores computed transposed S^T[k,q] so no transposes are ever needed;
    causal masking of diagonal chunks via identity-matmul accumulation of a
    precomputed additive mask; softmax without max-subtraction (scores are
    ~N(0,1)); exp on ScalarE directly from PSUM (scale fused); Z via a
    ones-column matmul; P@V accumulated into outT[d,q]; normalization by
    reciprocal + GPSIMD partition-broadcast + DVE multiply on eviction.
  Phase 3: out-projection from the SBUF-resident attnT tiles.
"""

import sys

if "/opt/trn_rl_repo" not in sys.path:
    sys.path.insert(0, "/opt/trn_rl_repo")

import numpy as np

import concourse.bass as bass
import concourse.bacc as bacc
import concourse.mybir as mybir
import concourse.tile as tile
from concourse.bass_utils import run_bass_kernel_spmd

F32 = mybir.dt.float32
F32R = mybir.dt.float32r

B, T, C = 4, 2048, 2048
H = 16            # total heads
HG = 8            # heads per core (group)
D = 128           # head dim
GC = HG * D       # channels per group = 1024
SCALE = 1.0 / float(np.sqrt(D))
MASKVAL = -30000.0
N_CORES = 8

KT = C // 128     # 16 K tiles
TT = T // 128     # 16 T tiles
TC = T // 512     # 4 T chunks of 512


def build_program(iters=1):
    nc = bacc.Bacc("TRN2", target_bir_lowering=False, debug=False)

    xT = nc.dram_tensor("xT", [C, T], F32R, kind="ExternalInput").ap()
    wq = nc.dram_tensor("wq", [C, GC], F32R, kind="ExternalInput").ap()
    wk = nc.dram_tensor("wk", [C, GC], F32R, kind="ExternalInput").ap()
    wv = nc.dram_tensor("wv", [C, GC], F32R, kind="ExternalInput").ap()
    bq = nc.dram_tensor("bq", [GC, 1], F32, kind="ExternalInput").ap()
    bk = nc.dram_tensor("bk", [GC, 1], F32, kind="ExternalInput").ap()
    bv = nc.dram_tensor("bv", [1, GC], F32R, kind="ExternalInput").ap()
    wo = nc.dram_tensor("wo", [GC, C], F32R, kind="ExternalInput").ap()
    sin2 = nc.dram_tensor("sin2", [128, T], F32, kind="ExternalInput").ap()
    cos2 = nc.dram_tensor("cos2", [128, T], F32, kind="ExternalInput").ap()
    masks = nc.dram_tensor("masks", [4, 128, 512], F32R, kind="ExternalInput").ap()
    ident = nc.dram_tensor("ident", [128, 128], F32R, kind="ExternalInput").ap()
    onescol = nc.dram_tensor("onescol", [128, 1], F32R, kind="ExternalInput").ap()
    ones128 = nc.dram_tensor("ones128", [1, 128], F32R, kind="ExternalInput").ap()
    y = nc.dram_tensor("y", [T, C], F32, kind="ExternalOutput").ap()

    with tile.TileContext(nc) as tc:
        with tc.tile_pool(name="dram", bufs=1, space="DRAM") as dpool, \
             tc.tile_pool(name="consts", bufs=1) as rpool:
            # DRAM scratch: unroped q,k rows (f32) and v (f32r)
            qk_d = [dpool.tile([128, T], F32, tag=f"qkd{m}", name=f"qkd{m}")
                    for m in range(16)]
            v_d = [dpool.tile([128, GC], F32R, tag=f"vd{t}", name=f"vd{t}")
                   for t in range(TT)]
            masks_sb = rpool.tile([128, 4 * 512], F32R, tag="masks")
            ident_sb = rpool.tile([128, 128], F32R, tag="ident")
            onescol_sb = rpool.tile([128, 1], F32R, tag="onescol")
            ones128_sb = rpool.tile([1, 128], F32R, tag="ones128")
            bv_sb = rpool.tile([1, GC], F32R, tag="bv")

            def full_body(iv):
                nc.sync.dma_start(
                    out=masks_sb[:].rearrange("p (r c) -> p r c", r=4),
                    in_=masks.rearrange("r p c -> p r c"),
                )
                nc.sync.dma_start(out=ident_sb[:], in_=ident)
                nc.sync.dma_start(out=onescol_sb[:], in_=onescol)
                nc.sync.dma_start(out=ones128_sb[:], in_=ones128)
                nc.sync.dma_start(out=bv_sb[:], in_=bv)

                # ---------------- Phase 1: QKV projection ----------------
                with tc.tile_pool(name="p1x", bufs=1) as xpool, \
                     tc.tile_pool(name="p1w", bufs=2) as wpool, \
                     tc.tile_pool(name="p1wv", bufs=1) as wvpool, \
                     tc.tile_pool(name="p1t", bufs=3) as tpool, \
                     tc.tile_pool(name="p1ps", bufs=2, space="PSUM") as pspool:
                    xt_sb = []
                    for k in range(KT):
                        t = xpool.tile([128, T], F32R, tag=f"xt{k}", name=f"xt{k}")
                        nc.sync.dma_start(out=t[:], in_=xT[k * 128:(k + 1) * 128, :])
                        xt_sb.append(t)

                    # q and k rows: 16 M-tiles of 128 output channels, [D,T] layout
                    for m in range(16):
                        w = wq if m < 8 else wk
                        bias = bq if m < 8 else bk
                        row = m % 8
                        wrow = wpool.tile([128, KT * 128], F32R, tag="wrow")
                        nc.sync.dma_start(
                            out=wrow[:].rearrange("p (k c) -> p k c", k=KT),
                            in_=w[:, row * 128:(row + 1) * 128].rearrange(
                                "(k p) c -> p k c", p=128
                            ),
                        )
                        bias_t = wpool.tile([128, 1], F32, tag="bias")
                        nc.sync.dma_start(
                            out=bias_t[:], in_=bias[row * 128:(row + 1) * 128, :]
                        )
                        for n in range(TC):
                            ps = pspool.tile([128, 512], F32, tag="pqk")
                            for k in range(KT):
                                nc.tensor.matmul(
                                    ps[:],
                                    wrow[:, k * 128:(k + 1) * 128],
                                    xt_sb[k][:, n * 512:(n + 1) * 512],
                                    start=(k == 0),
                                    stop=(k == KT - 1),
                                )
                            qt = tpool.tile([128, 512], F32, tag="qt")
                            nc.vector.tensor_scalar_add(qt[:], ps[:], bias_t[:])
                            nc.sync.dma_start(
                                out=qk_d[m][:, n * 512:(n + 1) * 512], in_=qt[:]
                            )

                    # v: [T, GC] untransposed, bias via K=1 matmul; N chunks of 256
                    for nd in range(4):
                        ndsl = slice(nd * 256, (nd + 1) * 256)
                        wvc = wvpool.tile([128, KT * 256], F32R, tag="wvc")
                        nc.sync.dma_start(
                            out=wvc[:].rearrange("p (k c) -> p k c", k=KT),
                            in_=wv[:, ndsl].rearrange("(k p) c -> p k c", p=128),
                        )
                        for t in range(TT):
                            ps = pspool.tile([128, 256], F32, tag="pv")
                            for k in range(KT):
                                nc.tensor.matmul(
                                    ps[:],
                                    xt_sb[k][:, t * 128:(t + 1) * 128],
                                    wvc[:, k * 256:(k + 1) * 256],
                                    start=(k == 0),
                                    stop=False,
                                )
                            nc.tensor.matmul(
                                ps[:],
                                ones128_sb[:],
                                bv_sb[:, ndsl],
                                start=False,
                                stop=True,
                            )
                            vt = tpool.tile([128, 256], F32R, tag="vt")
                            nc.scalar.copy(vt[:], ps[:])
                            nc.sync.dma_start(out=v_d[t][:, ndsl], in_=vt[:])

                # ---------------- Phases 2+3 ----------------
                with tc.tile_pool(name="p23attn", bufs=1) as apool:
                    attn_sb = [
                        apool.tile([128, T], F32R, tag=f"attn{h}", name=f"attn{h}")
                        for h in range(HG)
                    ]
                    sin_sb = apool.tile([128, T], F32, tag="sin")
                    cos_sb = apool.tile([128, T], F32, tag="cos")
                    nc.sync.dma_start(out=sin_sb[:], in_=sin2)
                    nc.sync.dma_start(out=cos_sb[:], in_=cos2)

                    # ---- Phase 2: attention per head ----
                    with tc.tile_pool(name="p2raw", bufs=1) as rawpool, \
                         tc.tile_pool(name="p2rope", bufs=1) as ropepool, \
                         tc.tile_pool(name="p2e", bufs=4) as epool, \
                         tc.tile_pool(name="p2n", bufs=2) as npool, \
                         tc.tile_pool(name="p2ps", bufs=2, space="PSUM") as ps2, \
                         tc.tile_pool(name="p2po", bufs=2, space="PSUM") as po2:
                        for h in range(HG):
                            qraw = rawpool.tile([128, T], F32, tag="qraw")
                            kraw = rawpool.tile([128, T], F32, tag="kraw")
                            qsw = rawpool.tile([128, T], F32, tag="qsw")
                            ksw = rawpool.tile([128, T], F32, tag="ksw")
                            vh = rawpool.tile([128, TT * 128], F32R, tag="vh")
                            nc.sync.dma_start(out=qraw[:], in_=qk_d[h][:])
                            nc.sync.dma_start(out=kraw[:], in_=qk_d[8 + h][:])
                            # partition-swapped copies (halves exchanged)
                            nc.sync.dma_start(out=qsw[0:64, :], in_=qk_d[h][64:128, :])
                            nc.sync.dma_start(out=qsw[64:128, :], in_=qk_d[h][0:64, :])
                            nc.sync.dma_start(
                                out=ksw[0:64, :], in_=qk_d[8 + h][64:128, :]
                            )
                            nc.sync.dma_start(
                                out=ksw[64:128, :], in_=qk_d[8 + h][0:64, :]
                            )
                            for t in range(TT):
                                nc.sync.dma_start(
                                    out=vh[:, t * 128:(t + 1) * 128],
                                    in_=v_d[t][:, h * 128:(h + 1) * 128],
                                )
                            # RoPE: ro = raw*cos2 + swapped(raw)*[-sin; +sin]
                            qr = ropepool.tile([128, T], F32R, tag="qr")
                            kr = ropepool.tile([128, T], F32R, tag="kr")
                            for raw, sw, ro in ((qraw, qsw, qr), (kraw, ksw, kr)):
                                s = ropepool.tile([128, T], F32, tag="ropes")
                                c = ropepool.tile([128, T], F32, tag="ropec")
                                nc.vector.tensor_mul(s[:], sw[:], sin_sb[:])
                                nc.vector.tensor_mul(c[:], raw[:], cos_sb[:])
                                nc.vector.tensor_add(ro[:], c[:], s[:])
                            for n in range(TC):
                                jmax = 4 * (n + 1)
                                ps_o = po2.tile([128, 512], F32, tag="po")
                                ps_z = po2.tile([1, 512], F32, tag="pz")
                                qsl = slice(n * 512, (n + 1) * 512)
                                # process k-tiles in pairs: two score chunks
                                # into one 2-bank psum tile, ONE 1024-wide exp
                                for jp in range(jmax // 2):
                                    ps_s = ps2.tile([128, 1024], F32, tag="ps")
                                    for u in range(2):
                                        j = 2 * jp + u
                                        half = slice(u * 512, (u + 1) * 512)
                                        diag = (j // 4) == n
                                        nc.tensor.matmul(
                                            ps_s[:, half],
                                            kr[:, j * 128:(j + 1) * 128],
                                            qr[:, qsl],
                                            start=True,
                                            stop=not diag,
                                        )
                                        if diag:
                                            r = j % 4
                                            nc.tensor.matmul(
                                                ps_s[:, half],
                                                ident_sb[:],
                                                masks_sb[:, r * 512:(r + 1) * 512],
                                                start=False,
                                                stop=True,
                                            )
                                    pexp = epool.tile([128, 1024], F32R, tag="pexp")
                                    nc.scalar.activation(
                                        pexp[:],
                                        ps_s[:],
                                        mybir.ActivationFunctionType.Exp,
                                        scale=SCALE,
                                    )
                                    for u in range(2):
                                        j = 2 * jp + u
                                        half = slice(u * 512, (u + 1) * 512)
                                        nc.tensor.matmul(
                                            ps_o[:],
                                            vh[:, j * 128:(j + 1) * 128],
                                            pexp[:, half],
                                            start=(j == 0),
                                            stop=(j == jmax - 1),
                                        )
                                        nc.tensor.matmul(
                                            ps_z[:],
                                            onescol_sb[:],
                                            pexp[:, half],
                                            start=(j == 0),
                                            stop=(j == jmax - 1),
                                        )
                                rz = npool.tile([1, 512], F32, tag="rz")
                                nc.vector.reciprocal(rz[:], ps_z[:])
                                rzb = npool.tile([128, 512], F32, tag="rzb")
                                nc.gpsimd.partition_broadcast(rzb[:], rz[:])
                                nc.vector.tensor_mul(
                                    attn_sb[h][:, n * 512:(n + 1) * 512],
                                    ps_o[:],
                                    rzb[:],
                                )

                    # ---- Phase 3: out projection ----
                    with tc.tile_pool(name="p3w", bufs=2) as wpool3, \
                         tc.tile_pool(name="p3t", bufs=3) as tpool3, \
                         tc.tile_pool(name="p3ps", bufs=2, space="PSUM") as ps3:
                        for n in range(4):
                            woc = wpool3.tile([128, HG * 512], F32R, tag="woc")
                            nc.sync.dma_start(
                                out=woc[:].rearrange("p (h c) -> p h c", h=HG),
                                in_=wo[:, n * 512:(n + 1) * 512].rearrange(
                                    "(h p) c -> p h c", p=128
                                ),
                            )
                            for m in range(TT):
                                ps_y = ps3.tile([128, 512], F32, tag="py")
                                for h in range(HG):
                                    nc.tensor.matmul(
                                        ps_y[:],
                                        attn_sb[h][:, m * 128:(m + 1) * 128],
                                        woc[:, h * 512:(h + 1) * 512],
                                        start=(h == 0),
                                        stop=(h == HG - 1),
                                    )
                                yt = tpool3.tile([128, 512], F32, tag="yt")
                                nc.scalar.copy(yt[:], ps_y[:])
                                nc.sync.dma_start(
                                    out=y[m * 128:(m + 1) * 128,
                                          n * 512:(n + 1) * 512],
                                    in_=yt[:],
                                )

            if iters == 1:
                full_body(None)
            else:
                with tc.For_i(0, iters, 1) as iv:
                    full_body(iv)

    nc.compile()
    return nc


def make_host_inputs(x, Wqkv, bqkv, Wo):
    """Per-core input maps (host-side sharding)."""
    half = D // 2
    freq = np.arange(half, dtype=np.float64)
    theta = 1.0 / (10000.0 ** (2.0 * freq / D))
    pos = np.arange(T, dtype=np.float64)
    ang = pos[:, None] * theta[None, :]          # [T, half]
    sinT = np.sin(ang).T.astype(np.float32)      # [half, T]
    cosT = np.cos(ang).T.astype(np.float32)
    # sign folded into the sin table for the partition-swap RoPE form
    sin2 = np.concatenate([-sinT, sinT], axis=0)  # [128, T]
    cos2 = np.concatenate([cosT, cosT], axis=0)

    masks = np.zeros((4, 128, 512), dtype=np.float32)
    f = np.arange(512)[None, :]
    p = np.arange(128)[:, None]
    for r in range(4):
        masks[r] = np.where(f >= r * 128 + p, 0.0, MASKVAL)
    ident = np.eye(128, dtype=np.float32)
    onescol = np.ones((128, 1), dtype=np.float32)
    ones128 = np.ones((1, 128), dtype=np.float32)

    xT = [np.ascontiguousarray(x[b].T) for b in range(B)]
    in_maps = []
    for core in range(N_CORES):
        b, g = core // 2, core % 2
        cs = slice(g * GC, (g + 1) * GC)
        in_maps.append({
            "xT": xT[b],
            "wq": np.ascontiguousarray(Wqkv[:, :C][:, cs]),
            "wk": np.ascontiguousarray(Wqkv[:, C:2 * C][:, cs]),
            "wv": np.ascontiguousarray(Wqkv[:, 2 * C:][:, cs]),
            "bq": np.ascontiguousarray(bqkv[:C][cs].reshape(GC, 1)),
            "bk": np.ascontiguousarray(bqkv[C:2 * C][cs].reshape(GC, 1)),
            "bv": np.ascontiguousarray(bqkv[2 * C:][cs].reshape(1, GC)),
            "wo": np.ascontiguousarray(Wo[cs, :]),
            "sin2": sin2,
            "cos2": cos2,
            "masks": masks,
            "ident": ident,
            "onescol": onescol,
            "ones128": ones128,
        })
    return in_maps


_PROGRAM_CACHE = {}


def get_program(iters=1):
    if iters not in _PROGRAM_CACHE:
        _PROGRAM_CACHE[iters] = build_program(iters)
    return _PROGRAM_CACHE[iters]


def kernel(x, Wqkv, bqkv, Wo, bo):
    x = np.asarray(x, dtype=np.float32)
    Wqkv = np.asarray(Wqkv, dtype=np.float32)
    bqkv = np.asarray(bqkv, dtype=np.float32)
    Wo = np.asarray(Wo, dtype=np.float32)
    bo = np.asarray(bo, dtype=np.float32)

    nc = get_program(1)
    in_maps = make_host_inputs(x, Wqkv, bqkv, Wo)
    res = run_bass_kernel_spmd(nc, in_maps, list(range(N_CORES)))

    out = np.empty((B, T, C), dtype=np.float32)
    for b in range(B):
        out[b] = res.results[2 * b]["y"] + res.results[2 * b + 1]["y"] + bo
    return out



# revision 3
# speedup vs baseline: 2.1197x; 2.1197x over previous
"""nn_MultiHeadAttention kernel for 8 Trainium2 NeuronCores.

Sharding: 8 cores = 4 batches (data parallel) x 2 head-groups of 8 heads
(tensor parallel). Each core computes its batch's QKV projection for its
head group (column-parallel), RoPE, causal attention, and a partial
out-projection (row-parallel). Host sums the two partials per batch and
adds the output bias.

v2 design (vs the DRAM-spill baseline):
  - All matmul operands bf16 (host pre-converts); fp32 PSUM accumulation.
  - Everything SBUF-resident: x (8MB), q/k rows (8MB, roped in place),
    v (4MB), attn (4MB). No DRAM scratch roundtrips.
  - Weights host-prearranged to [128, ...] layouts so every weight DMA is
    a contiguous slice (4KB/partition lines).
  - Phase order: V-proj (with the first Q tile interleaved to hide the wv
    chunk reload), Q/K-proj (head-major), per-head attention, out-proj.
    RoPE runs in place on the q/k tiles; RoPE for head h+2 is emitted
    during head h so the PE never waits on DVE.
  - Per q-chunk, score matmuls run one j-pair ahead of the PV/Z matmuls so
    the ScalarE exp never stalls the PE (keeps the HAM clock gate at 8/8).
  - Scores stay transposed S^T[k,q]: softmax denominator via a ones-column
    matmul, mask via identity-matmul accumulation of an additive mask,
    exp on ScalarE straight from PSUM with the 1/sqrt(D) scale fused,
    normalization via DVE reciprocal + GPSIMD broadcast + DVE multiply.
"""

import sys

if "/opt/trn_rl_repo" not in sys.path:
    sys.path.insert(0, "/opt/trn_rl_repo")

import numpy as np
import ml_dtypes

import concourse.bass as bass
import concourse.bacc as bacc
import concourse.mybir as mybir
import concourse.tile as tile
from concourse.bass_utils import run_bass_kernel_spmd

F32 = mybir.dt.float32
BF16 = mybir.dt.bfloat16
BF_NP = ml_dtypes.bfloat16

B, T, C = 4, 2048, 2048
H = 16            # total heads
HG = 8            # heads per core (group)
D = 128           # head dim
GC = HG * D       # channels per group = 1024
SCALE = 1.0 / float(np.sqrt(D))
MASKVAL = -30000.0
N_CORES = 8

KT = C // 128     # 16 contraction tiles
TT = T // 128     # 16 T tiles
TC = T // 512     # 4 T chunks of 512
ND = 2            # v output chunks of 512


def build_program(iters=1):
    nc = bacc.Bacc("TRN2", target_bir_lowering=False, debug=False)

    xT = nc.dram_tensor("xT", [C, T], BF16, kind="ExternalInput").ap()
    # m-major: [p=128, m(16: q0..q7,k0..k7), k(16), c(128)]
    wqk = nc.dram_tensor("wqk", [128, 16 * KT * 128], BF16,
                         kind="ExternalInput").ap()
    bqk = nc.dram_tensor("bqk", [2 * GC, 1], F32, kind="ExternalInput").ap()
    # nd-major: [p=128, nd(2), k(16), c(512)]
    wv = nc.dram_tensor("wv", [128, ND * KT * 512], BF16,
                        kind="ExternalInput").ap()
    bv = nc.dram_tensor("bv", [1, GC], BF16, kind="ExternalInput").ap()
    # n-major: [p=128, n(4), h(8), c(512)]
    wo = nc.dram_tensor("wo", [128, 4 * HG * 512], BF16,
                        kind="ExternalInput").ap()
    sin2 = nc.dram_tensor("sin2", [128, T], BF16, kind="ExternalInput").ap()
    cos2 = nc.dram_tensor("cos2", [128, T], BF16, kind="ExternalInput").ap()
    masks = nc.dram_tensor("masks", [128, 4 * 512], BF16,
                           kind="ExternalInput").ap()
    ident = nc.dram_tensor("ident", [128, 128], BF16,
                           kind="ExternalInput").ap()
    onescol = nc.dram_tensor("onescol", [128, 1], BF16,
                             kind="ExternalInput").ap()
    ones128 = nc.dram_tensor("ones128", [1, 128], BF16,
                             kind="ExternalInput").ap()
    y = nc.dram_tensor("y", [T, C], F32, kind="ExternalOutput").ap()

    with tile.TileContext(nc) as tc:
        with tc.tile_pool(name="consts", bufs=1) as rpool, \
             tc.tile_pool(name="qkp", bufs=1) as qkpool, \
             tc.tile_pool(name="vp", bufs=1) as vpool, \
             tc.tile_pool(name="ropep", bufs=1) as ropepool:
            masks_sb = rpool.tile([128, 4 * 512], BF16, tag="masks")
            ident_sb = rpool.tile([128, 128], BF16, tag="ident")
            onescol_sb = rpool.tile([128, 1], BF16, tag="onescol")
            ones128_sb = rpool.tile([1, 128], BF16, tag="ones128")
            bv_sb = rpool.tile([1, GC], BF16, tag="bv")
            sin_sb = rpool.tile([128, T], BF16, tag="sin")
            cos_sb = rpool.tile([128, T], BF16, tag="cos")

            qk_sb = [qkpool.tile([128, T], BF16, tag=f"qk{m}", name=f"qk{m}")
                     for m in range(16)]
            v_sb = [vpool.tile([128, GC], BF16, tag=f"v{t}", name=f"v{t}")
                    for t in range(TT)]

            def emit_rope(h):
                # in-place RoPE on qk_sb[h] (q) and qk_sb[8+h] (k):
                #   roped = raw * cos2 + swapped(raw) * sin2
                for m in (h, 8 + h):
                    sw = ropepool.tile([128, T], BF16, tag="sw", bufs=1,
                                       name=f"sw{m}")
                    nc.sync.dma_start(out=sw[0:64, :], in_=qk_sb[m][64:128, :])
                    nc.sync.dma_start(out=sw[64:128, :], in_=qk_sb[m][0:64, :])
                    tmp = ropepool.tile([128, T], BF16, tag="tmp", bufs=1,
                                        name=f"tmp{m}")
                    nc.vector.tensor_mul(tmp[:], sw[:], sin_sb[:])
                    nc.vector.tensor_mul(sw[:], qk_sb[m][:], cos_sb[:])
                    nc.vector.tensor_add(qk_sb[m][:], tmp[:], sw[:])

            def full_body(iv):
                nc.sync.dma_start(out=masks_sb[:], in_=masks)
                nc.sync.dma_start(out=ident_sb[:], in_=ident)
                nc.sync.dma_start(out=onescol_sb[:], in_=onescol)
                nc.sync.dma_start(out=ones128_sb[:], in_=ones128)
                nc.sync.dma_start(out=bv_sb[:], in_=bv)
                nc.sync.dma_start(out=sin_sb[:], in_=sin2)
                nc.sync.dma_start(out=cos_sb[:], in_=cos2)

                # ---------- Phases V + QK (x resident, weights streamed) ----
                with tc.tile_pool(name="xp", bufs=1) as xpool, \
                     tc.tile_pool(name="wvp", bufs=1) as wvpool, \
                     tc.tile_pool(name="w1p", bufs=1) as w1pool, \
                     tc.tile_pool(name="ps1", bufs=1, space="PSUM") as ps1:
                    xt_sb = []
                    for k in range(KT):
                        t = xpool.tile([128, T], BF16, tag=f"xt{k}",
                                       name=f"xt{k}")
                        nc.sync.dma_start(out=t[:],
                                          in_=xT[k * 128:(k + 1) * 128, :])
                        xt_sb.append(t)

                    def emit_v_chunk(nd):
                        # v[:, nd*512:(nd+1)*512] = x @ Wv chunk + bias
                        wvc = wvpool.tile([128, KT * 512], BF16, tag="wvc",
                                          bufs=1, name=f"wvc{nd}")
                        nc.sync.dma_start(
                            out=wvc[:],
                            in_=wv[:, nd * KT * 512:(nd + 1) * KT * 512])
                        ndsl = slice(nd * 512, (nd + 1) * 512)
                        for t in range(TT):
                            ps = ps1.tile([128, 512], F32, tag="ps1", bufs=4,
                                          name=f"psv{nd}_{t}")
                            for k in range(KT):
                                nc.tensor.matmul(
                                    ps[:],
                                    xt_sb[k][:, t * 128:(t + 1) * 128],
                                    wvc[:, k * 512:(k + 1) * 512],
                                    start=(k == 0), stop=False)
                            nc.tensor.matmul(
                                ps[:], ones128_sb[:], bv_sb[:, ndsl],
                                start=False, stop=True)
                            nc.scalar.copy(v_sb[t][:, ndsl], ps[:])

                    def emit_qk_tile(m):
                        # qk_sb[m] = (x^T @ Wqk col-block m)^T + bias  ([d, T])
                        wrow = w1pool.tile([128, KT * 128], BF16, tag="wrow",
                                           bufs=2, name=f"wrow{m}")
                        nc.sync.dma_start(
                            out=wrow[:], in_=wqk[:, m * 2048:(m + 1) * 2048])
                        bias_t = w1pool.tile([128, 1], F32, tag="bias",
                                             bufs=2, name=f"bias{m}")
                        nc.sync.dma_start(
                            out=bias_t[:], in_=bqk[m * 128:(m + 1) * 128, :])
                        for n in range(TC):
                            ps = ps1.tile([128, 512], F32, tag="ps1", bufs=4,
                                          name=f"psqk{m}_{n}")
                            for k in range(KT):
                                nc.tensor.matmul(
                                    ps[:],
                                    wrow[:, k * 128:(k + 1) * 128],
                                    xt_sb[k][:, n * 512:(n + 1) * 512],
                                    start=(k == 0), stop=(k == KT - 1))
                            nc.vector.tensor_scalar_add(
                                qk_sb[m][:, n * 512:(n + 1) * 512],
                                ps[:], bias_t[:])

                    emit_v_chunk(0)
                    emit_qk_tile(0)       # hides the wvc reload for nd=1
                    emit_v_chunk(1)
                    emit_qk_tile(8)
                    emit_rope(0)
                    for h in range(1, HG):
                        emit_qk_tile(h)
                        emit_qk_tile(8 + h)
                        if h == 1:
                            emit_rope(1)

                # ---------------- Phases 2+3 ----------------
                with tc.tile_pool(name="attnp", bufs=1) as apool:
                    attn_sb = [
                        apool.tile([128, T], BF16, tag=f"at{h}", name=f"at{h}")
                        for h in range(HG)
                    ]

                    # ---- Phase 2: attention per head ----
                    with tc.tile_pool(name="pexpp", bufs=1) as epool, \
                         tc.tile_pool(name="normp", bufs=1) as npool, \
                         tc.tile_pool(name="ps2s", bufs=2,
                                      space="PSUM") as ps2, \
                         tc.tile_pool(name="ps2o", bufs=2,
                                      space="PSUM") as po2, \
                         tc.tile_pool(name="ps2z", bufs=2,
                                      space="PSUM") as pz2:
                        for h in range(HG):
                            if h + 2 < HG:
                                emit_rope(h + 2)
                            qr = qk_sb[h]
                            kr = qk_sb[8 + h]
                            for n in range(TC):
                                jmax = 4 * (n + 1)
                                qsl = slice(n * 512, (n + 1) * 512)
                                ps_o = po2.tile([128, 512], F32, tag="po",
                                                name=f"po{h}_{n}")
                                ps_z = pz2.tile([1, 512], F32, tag="pz",
                                                name=f"pz{h}_{n}")

                                def emit_pv(jp, pexp, ps_o=ps_o, ps_z=ps_z,
                                            jmax=jmax, h=h):
                                    for u in range(2):
                                        j = 2 * jp + u
                                        half = slice(u * 512, (u + 1) * 512)
                                        nc.tensor.matmul(
                                            ps_o[:],
                                            v_sb[j][:, h * 128:(h + 1) * 128],
                                            pexp[:, half],
                                            start=(j == 0),
                                            stop=(j == jmax - 1))
                                        nc.tensor.matmul(
                                            ps_z[:], onescol_sb[:],
                                            pexp[:, half],
                                            start=(j == 0),
                                            stop=(j == jmax - 1))

                                prev = None
                                for jp in range(jmax // 2):
                                    ps_s = ps2.tile([128, 1024], F32,
                                                    tag="ps",
                                                    name=f"ps{h}_{n}_{jp}")
                                    for u in range(2):
                                        j = 2 * jp + u
                                        half = slice(u * 512, (u + 1) * 512)
                                        diag = (j // 4) == n
                                        nc.tensor.matmul(
                                            ps_s[:, half],
                                            kr[:, j * 128:(j + 1) * 128],
                                            qr[:, qsl],
                                            start=True, stop=not diag)
                                        if diag:
                                            r = j % 4
                                            nc.tensor.matmul(
                                                ps_s[:, half], ident_sb[:],
                                                masks_sb[:, r * 512:
                                                         (r + 1) * 512],
                                                start=False, stop=True)
                                    pexp = epool.tile(
                                        [128, 1024], BF16, tag="pexp", bufs=4,
                                        name=f"pexp{h}_{n}_{jp}")
                                    nc.scalar.activation(
                                        pexp[:], ps_s[:],
                                        mybir.ActivationFunctionType.Exp,
                                        scale=SCALE)
                                    if prev is not None:
                                        emit_pv(*prev)
                                    prev = (jp, pexp)
                                emit_pv(*prev)

                                rz = npool.tile([1, 512], F32, tag="rz",
                                                bufs=2, name=f"rz{h}_{n}")
                                nc.vector.reciprocal(rz[:], ps_z[:])
                                rzb = npool.tile([128, 512], F32, tag="rzb",
                                                 bufs=2, name=f"rzb{h}_{n}")
                                nc.gpsimd.partition_broadcast(rzb[:], rz[:])
                                nc.vector.tensor_mul(
                                    attn_sb[h][:, qsl], ps_o[:], rzb[:])

                    # ---- Phase 3: out projection ----
                    with tc.tile_pool(name="w3p", bufs=1) as w3pool, \
                         tc.tile_pool(name="yp", bufs=1) as ypool, \
                         tc.tile_pool(name="ps3", bufs=1,
                                      space="PSUM") as ps3:
                        for n in range(4):
                            woc = w3pool.tile([128, HG * 512], BF16,
                                              tag="woc", bufs=2,
                                              name=f"woc{n}")
                            nc.sync.dma_start(
                                out=woc[:],
                                in_=wo[:, n * HG * 512:(n + 1) * HG * 512])
                            for m in range(TT):
                                ps_y = ps3.tile([128, 512], F32, tag="py",
                                                bufs=4, name=f"py{n}_{m}")
                                for h in range(HG):
                                    nc.tensor.matmul(
                                        ps_y[:],
                                        attn_sb[h][:, m * 128:(m + 1) * 128],
                                        woc[:, h * 512:(h + 1) * 512],
                                        start=(h == 0), stop=(h == HG - 1))
                                yt = ypool.tile([128, 512], F32, tag="yt",
                                                bufs=3, name=f"yt{n}_{m}")
                                nc.scalar.copy(yt[:], ps_y[:])
                                nc.sync.dma_start(
                                    out=y[m * 128:(m + 1) * 128,
                                          n * 512:(n + 1) * 512],
                                    in_=yt[:])

            if iters == 1:
                full_body(None)
            else:
                with tc.For_i(0, iters, 1) as iv:
                    full_body(iv)

    nc.compile()
    return nc


def make_host_inputs(x, Wqkv, bqkv, Wo):
    """Per-core input maps (host-side sharding + bf16 conversion)."""
    half = D // 2
    freq = np.arange(half, dtype=np.float64)
    theta = 1.0 / (10000.0 ** (2.0 * freq / D))
    pos = np.arange(T, dtype=np.float64)
    ang = pos[:, None] * theta[None, :]          # [T, half]
    sinT = np.sin(ang).T.astype(np.float32)      # [half, T]
    cosT = np.cos(ang).T.astype(np.float32)
    # sign folded into the sin table for the partition-swap RoPE form
    sin2 = np.concatenate([-sinT, sinT], axis=0).astype(BF_NP)  # [128, T]
    cos2 = np.concatenate([cosT, cosT], axis=0).astype(BF_NP)

    masks = np.zeros((4, 128, 512), dtype=np.float32)
    f = np.arange(512)[None, :]
    p = np.arange(128)[:, None]
    for r in range(4):
        masks[r] = np.where(f >= r * 128 + p, 0.0, MASKVAL)
    # [4,128,512] -> [128, 4*512]
    masks = np.ascontiguousarray(
        masks.transpose(1, 0, 2).reshape(128, 4 * 512)).astype(BF_NP)
    ident = np.eye(128, dtype=np.float32).astype(BF_NP)
    onescol = np.ones((128, 1), dtype=np.float32).astype(BF_NP)
    ones128 = np.ones((1, 128), dtype=np.float32).astype(BF_NP)

    xT = [np.ascontiguousarray(x[b].T).astype(BF_NP) for b in range(B)]
    in_maps = []
    for core in range(N_CORES):
        b, g = core // 2, core % 2
        cs = slice(g * GC, (g + 1) * GC)
        Wq = Wqkv[:, :C][:, cs]
        Wk = Wqkv[:, C:2 * C][:, cs]
        Wv = Wqkv[:, 2 * C:][:, cs]
        # [C, 2*GC] -> [p, m, k, c] -> [128, 16*16*128]
        Wqk = np.concatenate([Wq, Wk], axis=1)
        wqk_r = np.ascontiguousarray(
            Wqk.reshape(KT, 128, 16, 128).transpose(1, 2, 0, 3)
            .reshape(128, 16 * KT * 128)).astype(BF_NP)
        bqk_r = np.concatenate(
            [bqkv[:C][cs], bqkv[C:2 * C][cs]]).reshape(2 * GC, 1)
        bqk_r = np.ascontiguousarray(bqk_r).astype(np.float32)
        # [C, GC] -> [p, nd, k, c] -> [128, 2*16*512]
        wv_r = np.ascontiguousarray(
            Wv.reshape(KT, 128, ND, 512).transpose(1, 2, 0, 3)
            .reshape(128, ND * KT * 512)).astype(BF_NP)
        bv_r = np.ascontiguousarray(
            bqkv[2 * C:][cs].reshape(1, GC)).astype(BF_NP)
        # [GC, C] -> [p, n, h, c] -> [128, 4*8*512]
        wo_r = np.ascontiguousarray(
            Wo[cs, :].reshape(HG, 128, 4, 512).transpose(1, 2, 0, 3)
            .reshape(128, 4 * HG * 512)).astype(BF_NP)
        in_maps.append({
            "xT": xT[b],
            "wqk": wqk_r,
            "bqk": bqk_r,
            "wv": wv_r,
            "bv": bv_r,
            "wo": wo_r,
            "sin2": sin2,
            "cos2": cos2,
            "masks": masks,
            "ident": ident,
            "onescol": onescol,
            "ones128": ones128,
        })
    return in_maps


_PROGRAM_CACHE = {}


def get_program(iters=1):
    if iters not in _PROGRAM_CACHE:
        _PROGRAM_CACHE[iters] = build_program(iters)
    return _PROGRAM_CACHE[iters]


def kernel(x, Wqkv, bqkv, Wo, bo):
    x = np.asarray(x, dtype=np.float32)
    Wqkv = np.asarray(Wqkv, dtype=np.float32)
    bqkv = np.asarray(bqkv, dtype=np.float32)
    Wo = np.asarray(Wo, dtype=np.float32)
    bo = np.asarray(bo, dtype=np.float32)

    nc = get_program(1)
    in_maps = make_host_inputs(x, Wqkv, bqkv, Wo)
    res = run_bass_kernel_spmd(nc, in_maps, list(range(N_CORES)))

    out = np.empty((B, T, C), dtype=np.float32)
    for b in range(B):
        out[b] = res.results[2 * b]["y"] + res.results[2 * b + 1]["y"] + bo
    return out
